# revision 1
# baseline (speedup 1.0000x reference)
"""CrossAttentionS2T (attn_all_frame=True) as a Bass/Tile kernel on 8 trn2 cores.

Strategy: data-parallel over batch B=8 -> one batch element per NeuronCore.
Per core, all activations live in transposed [feature, token] layout so every
matmul contracts over the partition axis at full 128-wide PE utilization:

  q_in.T [768,1568]   = t_x slice.T + pos (device add)
  s.T    [768, 784]   = s_x slice.T + pos (device add)
  q.T  = (0.125*Wq) @ q_in.T + 0.125*qb      (scale folded into weights: exact)
  k.T  = Wk @ s.T + kb ; v (natural) = s.T.T @ Wv.T + vb
  scores.T[k,q] = k_h.T^T-contraction -> exp (no max-sub; scores are O(1))
  [o_unnorm.T ; denom] = [v_h | 1]^T @ probs.T   (ones column => softmax denom)
  o.T = o_unnorm.T * bcast(1/denom)
  out.T = Wproj @ o.T + pb -> PE-transpose -> natural [1568,768] -> DMA out

Matmul inputs are bitcast to float32r (full fp32 data, 1 cycle/row for moving
free dim >= 256 on trn2 vs 4 cycles/row for plain fp32).
"""

import math
import os
from contextlib import ExitStack

import numpy as np

import concourse.bass as bass
import concourse.mybir as mybir
import concourse.tile as tile
from concourse.bass import ds, ts
from concourse.masks import make_identity

F32 = mybir.dt.float32
F32R = mybir.dt.float32r
AF = mybir.ActivationFunctionType

# problem dims (hardcoded per contract)
B, SPEC, T = 8, 4, 8
AP_, VP, DIM = 196, 196, 768
NH, HD = 12, 64
SCALE = HD ** -0.5
NQ = VP * T          # 1568 q tokens per batch
NK = AP_ * SPEC      # 784 kv tokens per batch
DC = DIM // 128      # 6 contraction chunks
QT, NQT = 392, 4     # q-token tile (moving free dim)
KB, NKB = 112, 7     # k-token block (scores.T partition dim)
VW, NVW = 384, 2     # v feature tile for natural-layout V projection
NCORES = 8


def _r(ap):
    return ap.bitcast(F32R)


def _emit(ctx, tc, outs, ins):
    nc = tc.nc
    (txT, sxT, posqT, possT, wqT, wkT, wvT, wpT, qb2, kb2, pb2, vbb, ones1) = ins
    (out_d,) = outs

    const = ctx.enter_context(tc.tile_pool(name="const", bufs=1))
    ident = const.tile([128, 128], F32)
    make_identity(nc, ident)
    qb_t = const.tile([128, DC], F32)
    kb_t = const.tile([128, DC], F32)
    pb_t = const.tile([128, DC], F32)
    vbb_t = const.tile([128, DIM], F32)
    nc.sync.dma_start(qb_t[:], qb2[:])
    nc.sync.dma_start(kb_t[:], kb2[:])
    nc.sync.dma_start(pb_t[:], pb2[:])
    nc.sync.dma_start(vbb_t[:], vbb[:])
    ones_t = const.tile([1, HD], F32)
    nc.sync.dma_start(_r(ones_t[:]), _r(ones1[:]))

    # persistent activations
    pers = ctx.enter_context(tc.tile_pool(name="pers", bufs=1))
    oT = [pers.tile([128, NQ], F32, name=f"oT{c}", tag=f"oT{c}") for c in range(DC)]
    kTt = [pers.tile([128, NK], F32, name=f"kT{c}", tag=f"kT{c}") for c in range(DC)]
    v_t = [pers.tile([KB, NH * (HD + 1)], F32, name=f"v{j}", tag=f"v{j}")
           for j in range(NKB)]
    qTt = [pers.tile([128, NQ], F32, name=f"qT{c}", tag=f"qT{c}") for c in range(DC)]
    wp_t = [pers.tile([128, DIM], F32, name=f"wp{c}", tag=f"wp{c}") for c in range(DC)]
    for c in range(DC):
        nc.sync.dma_start(_r(wp_t[c][:]), _r(wpT[ts(c, 128), :]))

    # PSUM pools: proj (3 banks) + attention qk (3) + o (2) = 8 banks total
    psA = ctx.enter_context(tc.tile_pool(name="psA", bufs=2, space="PSUM"))
    psB = ctx.enter_context(tc.tile_pool(name="psB", bufs=1, space="PSUM"))

    # ---- phase A: s.T build + KV projections ----
    with tc.tile_pool(name="phA", bufs=1) as phA:
        wk_t = [phA.tile([128, DIM], F32, name=f"wk{c}", tag=f"wk{c}")
                for c in range(DC)]
        wv_t = [phA.tile([128, DIM], F32, name=f"wv{c}", tag=f"wv{c}")
                for c in range(DC)]
        sT = [phA.tile([128, NK], F32, name=f"sT{c}", tag=f"sT{c}")
              for c in range(DC)]
        for c in range(DC):
            nc.sync.dma_start(_r(wk_t[c][:]), _r(wkT[ts(c, 128), :]))
            nc.sync.dma_start(_r(wv_t[c][:]), _r(wvT[ts(c, 128), :]))
        for c in range(DC):
            sx_t = phA.tile([128, NK], F32, name="sx_t", tag="ldA", bufs=2)
            nc.sync.dma_start(sx_t[:], sxT[ts(c, 128), :])
            ps_t = phA.tile([128, NK], F32, name="ps_t", tag="ldB", bufs=2)
            nc.sync.dma_start(ps_t[:], possT[ts(c, 128), :])
            nc.vector.tensor_add(_r(sT[c][:]), sx_t[:], ps_t[:])

        # K projection, transposed output layout [kfeat, ktok]
        for f in range(DC):
            for kt in range(2):
                ps = psA.tile([128, QT], F32, name="ps_k", tag="proj")
                for c in range(DC):
                    nc.tensor.matmul(
                        ps[:], _r(wk_t[c][:, ts(f, 128)]),
                        _r(sT[c][:, ts(kt, QT)]),
                        start=(c == 0), stop=(c == DC - 1))
                nc.scalar.activation(_r(kTt[f][:, ts(kt, QT)]), ps[:], AF.Identity,
                                     bias=kb_t[:, ds(f, 1)])

        # V projection, natural layout [ktok, vfeat], +1s column per head
        # (memset is not encodable with an f32r output; round via tensor_copy)
        vinit = phA.tile([KB, NH * (HD + 1)], F32, name="vinit", tag="vinit")
        nc.vector.memset(vinit[:], 1.0)
        for j in range(NKB):
            nc.vector.tensor_copy(_r(v_t[j][:]), vinit[:])
            for w in range(NVW):
                ps = psA.tile([KB, VW], F32, name="ps_v", tag="proj")
                for c in range(DC):
                    nc.tensor.matmul(
                        ps[:], _r(sT[c][:, ts(j, KB)]),
                        _r(wv_t[c][:, ts(w, VW)]),
                        start=(c == 0), stop=(c == DC - 1))
                for hh in range(6):
                    h = w * 6 + hh
                    nc.vector.tensor_add(
                        _r(v_t[j][:, ds(h * (HD + 1), HD)]),
                        ps[:, ts(hh, HD)],
                        vbb_t[0:KB, ds(w * VW + hh * HD, HD)])

    # ---- phase B: q_in.T build + Q projection (streamed per q-tile) ----
    with tc.tile_pool(name="phB", bufs=1) as phB:
        wq_t = [phB.tile([128, DIM], F32, name=f"wq{c}", tag=f"wq{c}")
                for c in range(DC)]
        for c in range(DC):
            nc.sync.dma_start(_r(wq_t[c][:]), _r(wqT[ts(c, 128), :]))
        for qt in range(NQT):
            qins = []
            for c in range(DC):
                tx_t = phB.tile([128, QT], F32, name="tx_t", tag="ldq", bufs=3)
                nc.gpsimd.dma_start(tx_t[:], txT[ts(c, 128), ts(qt, QT)])
                pq_t = phB.tile([128, QT], F32, name="pq_t", tag="ldp", bufs=3)
                nc.gpsimd.dma_start(pq_t[:], posqT[ts(c, 128), ts(qt, QT)])
                qin_c = phB.tile([128, QT], F32, name="qin", tag="qin", bufs=2 * DC)
                nc.vector.tensor_add(_r(qin_c[:]), tx_t[:], pq_t[:])
                qins.append(qin_c)
            for f in range(DC):
                ps = psA.tile([128, QT], F32, name="ps_q", tag="proj")
                for c in range(DC):
                    nc.tensor.matmul(
                        ps[:], _r(wq_t[c][:, ts(f, 128)]), _r(qins[c][:]),
                        start=(c == 0), stop=(c == DC - 1))
                nc.scalar.activation(_r(qTt[f][:, ts(qt, QT)]), ps[:], AF.Identity,
                                     bias=qb_t[:, ds(f, 1)])

    # ---- phase C: attention ----
    with tc.tile_pool(name="phC", bufs=1) as phC:
        for h in range(NH):
            ch, off = h // 2, (h % 2) * HD
            for qt in range(NQT):
                q_ap = qTt[ch][ds(off, HD), ts(qt, QT)]
                o_ps = psB.tile([HD + 1, QT], F32, name="o_ps", tag="o", bufs=3)
                probs = []
                for j in range(NKB):
                    s_ps = psB.tile([KB, QT], F32, name="s_ps", tag="qk", bufs=3)
                    nc.tensor.matmul(s_ps[:],
                                     _r(kTt[ch][ds(off, HD), ts(j, KB)]),
                                     _r(q_ap), start=True, stop=True)
                    p_t = phC.tile([KB, QT], F32, name="p_t", tag="probs", bufs=9)
                    nc.scalar.activation(_r(p_t[:]), s_ps[:], AF.Exp)
                    probs.append(p_t)
                for j in range(NKB):
                    nc.tensor.matmul(o_ps[:],
                                     _r(v_t[j][:, ds(h * (HD + 1), HD + 1)]),
                                     _r(probs[j][:]),
                                     start=(j == 0), stop=(j == NKB - 1))
                r1 = phC.tile([1, QT], F32R, name="r1", tag="r1", bufs=2)
                with nc.allow_low_precision(reason="f32r recip for bcast"):
                    nc.vector.reciprocal(r1[:], o_ps[ds(HD, 1), :])
                rb_ps = psB.tile([HD, QT], F32, name="rb_ps", tag="qk", bufs=3)
                nc.tensor.matmul(rb_ps[:], _r(ones_t[:]), r1[:],
                                 start=True, stop=True)
                rb = phC.tile([HD, QT], F32, name="rb", tag="rb", bufs=2)
                nc.vector.tensor_copy(rb[:], rb_ps[:])
                nc.vector.tensor_mul(_r(oT[ch][ds(off, HD), ts(qt, QT)]),
                                     o_ps[0:HD, :], rb[:])

    # ---- phase D: output projection + transpose to natural + DMA out ----
    with tc.tile_pool(name="phD", bufs=1) as phD:
        outT = [phD.tile([128, NQ], F32, name=f"outT{c}", tag=f"outT{c}")
                for c in range(DC)]
        for f in range(DC):
            for qt in range(NQT):
                ps = psA.tile([128, QT], F32, name="ps_o", tag="proj")
                for c in range(DC):
                    nc.tensor.matmul(
                        ps[:], _r(wp_t[c][:, ts(f, 128)]),
                        _r(oT[c][:, ts(qt, QT)]),
                        start=(c == 0), stop=(c == DC - 1))
                nc.scalar.activation(outT[f][:, ts(qt, QT)], ps[:], AF.Identity,
                                     bias=pb_t[:, ds(f, 1)])
        nblk = math.ceil(NQ / 128)  # 13 blocks: 12x128 + 32
        for qb in range(nblk):
            qw = min(128, NQ - qb * 128)
            o_nat = phD.tile([128, DIM], F32, name="o_nat", tag="onat", bufs=2)
            for f in range(DC):
                tp = psA.tile([128, 128], F32, name="tp", tag="proj")
                nc.tensor.transpose(tp[0:qw, :], outT[f][:, ds(qb * 128, qw)],
                                    ident[:])
                nc.vector.tensor_copy(o_nat[0:qw, ts(f, 128)], tp[0:qw, :])
            nc.sync.dma_start(out_d[ds(qb * 128, qw), :], o_nat[0:qw, :])


def build_program():
    from concourse import bacc
    from concourse.compiler_utils import get_compiler_flags, set_compiler_flags
    flags = [f.replace("--enable-ldw-opt=false", "--enable-ldw-opt=true")
             for f in get_compiler_flags()]
    set_compiler_flags(flags)
    nc = bacc.Bacc("TRN2", target_bir_lowering=False, debug=False,
                   num_devices=NCORES)
    mk = lambda name, shape, out=False: nc.dram_tensor(
        name, shape, F32, kind="ExternalOutput" if out else "ExternalInput").ap()
    ins = [
        mk("txT", [DIM, NQ]), mk("sxT", [DIM, NK]),
        mk("posqT", [DIM, NQ]), mk("possT", [DIM, NK]),
        mk("wqT", [DIM, DIM]), mk("wkT", [DIM, DIM]),
        mk("wvT", [DIM, DIM]), mk("wpT", [DIM, DIM]),
        mk("qb2", [128, DC]), mk("kb2", [128, DC]), mk("pb2", [128, DC]),
        mk("vbb", [128, DIM]), mk("ones1", [1, HD]),
    ]
    outs = [mk("out", [NQ, DIM], out=True)]
    with tile.TileContext(nc) as tc:
        with ExitStack() as ctx:
            _emit(ctx, tc, outs, ins)
    nc.compile()
    return nc


def host_prep(inputs):
    """Host-side layout marshalling: slice per core, transpose to
    [feature, token], fold the attention scale into Wq (exact: 0.125 = 2^-3),
    pre-broadcast positional sums and biases."""
    f32 = np.float32
    g = {k: np.asarray(v, dtype=f32) for k, v in inputs.items()}
    t_pat = g["t_x"][1:]                      # (VP, B*T, D)
    s_x = g["s_x"]                            # (AP, B*SPEC, D)

    posq = (g["vmae_space_pos"][:, None, :] + g["vmae_temporal_pos"][None, :, :])
    posq = np.ascontiguousarray(posq.reshape(NQ, DIM).T)          # (D, NQ)
    poss = (g["clip_space_pos"][:, None, :] + g["clip_temporal_pos"][None, :, :])
    poss = np.ascontiguousarray(poss.reshape(NK, DIM).T)          # (D, NK)

    wqT = np.ascontiguousarray((SCALE * g["Wq"]).T)
    wkT = np.ascontiguousarray(g["Wkv"][:DIM].T)
    wvT = np.ascontiguousarray(g["Wkv"][DIM:].T)
    wpT = np.ascontiguousarray(g["Wproj"].T)
    qb2 = np.ascontiguousarray((SCALE * g["q_bias"]).reshape(DC, 128).T)
    kb2 = np.ascontiguousarray(g["kv_bias"][:DIM].reshape(DC, 128).T)
    pb2 = np.ascontiguousarray(g["proj_bias"].reshape(DC, 128).T)
    vbb = np.ascontiguousarray(np.tile(g["kv_bias"][DIM:], (128, 1)))

    shared = dict(posqT=posq, possT=poss, wqT=wqT, wkT=wkT, wvT=wvT, wpT=wpT,
                  qb2=qb2, kb2=kb2, pb2=pb2, vbb=vbb,
                  ones1=np.ones((1, HD), dtype=f32))
    in_maps = []
    for b in range(B):
        txT = np.ascontiguousarray(
            t_pat[:, b * T:(b + 1) * T, :].reshape(NQ, DIM).T)
        sxT = np.ascontiguousarray(
            s_x[:, b * SPEC:(b + 1) * SPEC, :].reshape(NK, DIM).T)
        in_maps.append(dict(txT=txT, sxT=sxT, **shared))
    return in_maps


def host_finish(results, t_x):
    o = np.stack([results[b]["out"] for b in range(B)])   # (B, NQ, D)
    o = o.reshape(B, VP, T, DIM).transpose(1, 0, 2, 3).reshape(VP, B * T, DIM)
    return np.concatenate([np.asarray(t_x, dtype=np.float32)[0:1], o], axis=0)


_NC = None


def kernel(**inputs):
    global _NC
    from concourse.bass_utils import run_bass_kernel_spmd
    if _NC is None:
        _NC = build_program()
    in_maps = host_prep(inputs)
    res = run_bass_kernel_spmd(_NC, in_maps, list(range(NCORES)))
    return host_finish(res.results, inputs["t_x"])



# revision 10
# speedup vs baseline: 1.6016x; 1.6016x over previous
"""CrossAttentionS2T (attn_all_frame=True) as a Bass/Tile kernel on 8 trn2 cores.

Strategy: data-parallel over batch B=8 -> one batch element per NeuronCore.
All activations live in transposed [feature, token] layout so every matmul
contracts over the partition axis; everything is bf16 (validated 1.3e-4 rel
err end-to-end vs the 2e-2 gate) so DMA volume is halved and the PE runs at
1 cycle/row.

Key performance structure vs the v1 kernel:
  - positional sums are folded into the inputs on the host (q_in, s built
    host-side), removing 7.2MB of DMA and all input-side vector adds
  - Q projection runs first with contraction-outer accumulation over 6 psum
    banks, so the first matmul needs only 1 weight chunk + 1 input chunk
    (~0.8MB of DMA) instead of the whole input
  - softmax denominator comes from 64 replicated ones-columns appended to V
    per head: the PV matmul yields [64 o-rows | 64 denom-rows] in one psum
    tile; normalization is then a 64-lane reciprocal_approx_fast + one
    tensor_mul on DVE, entirely off the PE critical path (v1 used a 1-lane
    reciprocal + PE broadcast matmul per tile = 2.6us serial each)
  - attention is software-pipelined: scores(t+1) are emitted before the
    PV-accumulate(t), so the PE never waits for the Exp activations and the
    HAM clock gate stays at 2.4GHz
  - output is written transposed [feature, token] and the final transpose is
    done on the host (frees ~78 PE transposes + DVE copies)
"""

import math
import os
from contextlib import ExitStack

import numpy as np

import concourse.bass as bass
import concourse.mybir as mybir
import concourse.tile as tile
from concourse.bass import ds, ts

F32 = mybir.dt.float32
BF16 = mybir.dt.bfloat16
AF = mybir.ActivationFunctionType

# problem dims (hardcoded per contract)
B, SPEC, T = 8, 4, 8
AP_, VP, DIM = 196, 196, 768
NH, HD = 12, 64
SCALE = HD ** -0.5
NQ = VP * T          # 1568 q tokens per batch
NK = AP_ * SPEC      # 784 kv tokens per batch
DC = DIM // 128      # 6 contraction chunks
QT, NQT = 392, 4     # q-token tile (moving free dim)
KB, NKB = 112, 7     # k-token block (scores.T partition dim)
VW, NVW = 384, 2     # v feature tile for natural-layout V projection
VH = 128             # per-head stride in v tiles: 64 v cols + 64 ones cols
NCORES = 8


def _emit(ctx, tc, outs, ins):
    nc = tc.nc
    (qinT_d, sT_d, wqT_d, wkT_d, wvT_d, wpT_d, qb2, kb2, pb2, vbb) = ins
    (out_d,) = outs

    const = ctx.enter_context(tc.tile_pool(name="const", bufs=1))
    qb_t = const.tile([128, DC], F32)
    kb_t = const.tile([128, DC], F32)
    pb_t = const.tile([128, DC], F32)
    vbb_t = const.tile([128, DIM], F32)
    nc.sync.dma_start(qb_t[:], qb2[:])
    nc.sync.dma_start(kb_t[:], kb2[:])
    nc.sync.dma_start(pb_t[:], pb2[:])
    nc.sync.dma_start(vbb_t[:], vbb[:])

    # persistent activations (all bf16)
    pers = ctx.enter_context(tc.tile_pool(name="pers", bufs=1))
    qTt = [pers.tile([128, NQ], BF16, name=f"qT{c}", tag=f"qT{c}")
           for c in range(DC)]
    kTt = [pers.tile([128, NK], BF16, name=f"kT{c}", tag=f"kT{c}")
           for c in range(DC)]
    v_t = [pers.tile([KB, NH * VH], BF16, name=f"v{j}", tag=f"v{j}")
           for j in range(NKB)]
    oT = [pers.tile([128, NQ], BF16, name=f"oT{c}", tag=f"oT{c}")
          for c in range(DC)]
    wp_t = [pers.tile([128, DIM], BF16, name=f"wp{c}", tag=f"wp{c}")
            for c in range(DC)]

    # ones columns of v tiles (denominator trick): v_t[j][:, h*128+64 : +128]
    for j in range(NKB):
        for h in range(NH):
            nc.vector.memset(v_t[j][:, ds(h * VH + HD, HD)], 1.0)

    with tc.tile_pool(name="phKV", bufs=1) as phKV:
        wk_t = [phKV.tile([128, DIM], BF16, name=f"wk{c}", tag=f"wk{c}")
                for c in range(DC)]
        wv_t = [phKV.tile([128, DIM], BF16, name=f"wv{c}", tag=f"wv{c}")
                for c in range(DC)]
        sTn = [phKV.tile([128, NK], BF16, name=f"sT{c}", tag=f"sT{c}")
               for c in range(DC)]
        # KV-side DMAs go on the gpsimd queue, in parallel with the Q-side
        # stream on the sync queue
        for c in range(DC):
            nc.gpsimd.dma_start(wk_t[c][:], wkT_d[ts(c, 128), :])
            nc.gpsimd.dma_start(sTn[c][:], sT_d[ts(c, 128), :])
        for c in range(DC):
            nc.gpsimd.dma_start(wv_t[c][:], wvT_d[ts(c, 128), :])

        # ---- phase Q: Q projection, contraction-outer for fast PE start ----
        with tc.tile_pool(name="phQ", bufs=1) as phQ, \
                tc.tile_pool(name="psQ", bufs=6, space="PSUM") as psQ:
            wq_t = [phQ.tile([128, DIM], BF16, name=f"wq{c}", tag=f"wq{c}")
                    for c in range(DC)]
            qin = [phQ.tile([128, NQ], BF16, name=f"qi{c}", tag=f"qi{c}")
                   for c in range(DC)]
            for c in range(DC):
                nc.sync.dma_start(wq_t[c][:], wqT_d[ts(c, 128), :])
                nc.sync.dma_start(qin[c][:], qinT_d[ts(c, 128), :])
            for c in range(DC):
                nc.sync.dma_start(wp_t[c][:], wpT_d[ts(c, 128), :])
            for qt in range(NQT):
                pss = [psQ.tile([128, QT], F32, name="ps_q", tag="q")
                       for _ in range(DC)]
                for c in range(DC):
                    for f in range(DC):
                        nc.tensor.matmul(
                            pss[f][:], wq_t[c][:, ts(f, 128)],
                            qin[c][:, ts(qt, QT)],
                            start=(c == 0), stop=(c == DC - 1))
                for f in range(DC):
                    nc.scalar.activation(qTt[f][:, ts(qt, QT)], pss[f][:],
                                         AF.Identity, bias=qb_t[:, ds(f, 1)])

        # ---- phase KV: K (transposed) and V (natural + ones) projections ----
        with tc.tile_pool(name="psKV", bufs=6, space="PSUM") as psKV:
            for kt in range(2):
                psk = [psKV.tile([128, QT], F32, name="ps_k", tag="kv")
                       for _ in range(DC)]
                for c in range(DC):
                    for f in range(DC):
                        nc.tensor.matmul(
                            psk[f][:], wk_t[c][:, ts(f, 128)],
                            sTn[c][:, ts(kt, QT)],
                            start=(c == 0), stop=(c == DC - 1))
                for f in range(DC):
                    nc.scalar.activation(kTt[f][:, ts(kt, QT)], psk[f][:],
                                         AF.Identity, bias=kb_t[:, ds(f, 1)])
            for j in range(NKB):
                for w in range(NVW):
                    ps = psKV.tile([KB, VW], F32, name="ps_v", tag="kv")
                    for c in range(DC):
                        nc.tensor.matmul(
                            ps[:], sTn[c][:, ts(j, KB)], wv_t[c][:, ts(w, VW)],
                            start=(c == 0), stop=(c == DC - 1))
                    with nc.allow_low_precision(reason="bf16 v store"):
                        for hh in range(6):
                            h = w * 6 + hh
                            nc.vector.tensor_add(
                                v_t[j][:, ds(h * VH, HD)],
                                ps[:, ts(hh, HD)],
                                vbb_t[0:KB, ds(w * VW + hh * HD, HD)])

    if os.environ.get("K_PHASES", "all") == "qkv":
        with tc.tile_pool(name="dbg", bufs=1) as dbg:
            dt = dbg.tile([128, NK], F32, name="dbg_t", tag="dbg_t")
            nc.scalar.activation(dt[:], kTt[0][:], AF.Identity)
            nc.sync.dma_start(out_d[0:128, 0:NK], dt[:])
        return

    # ---- phase C: attention, software-pipelined ----
    with tc.tile_pool(name="phC", bufs=1) as phC, \
            tc.tile_pool(name="psS", bufs=6, space="PSUM") as psS, \
            tc.tile_pool(name="psO", bufs=2, space="PSUM") as psO:

        def _drain(prev):
            h, qt, pts = prev
            ch, off = h // 2, (h % 2) * HD
            op = psO.tile([128, QT], F32, name="o_ps", tag="o")
            for j in range(NKB):
                nc.tensor.matmul(op[:], v_t[j][:, ds(h * VH, VH)], pts[j][:],
                                 start=(j == 0), stop=(j == NKB - 1))
            scr = phC.tile([HD, QT], F32, name="scr", tag="scr", bufs=2)
            nc.vector.reciprocal(scr[:], op[ds(HD, HD), :])
            with nc.allow_low_precision(reason="bf16 attn out"):
                nc.vector.tensor_mul(oT[ch][ds(off, HD), ts(qt, QT)],
                                     op[ds(0, HD), :], scr[:])

        prev = None
        for h in range(NH):
            ch, off = h // 2, (h % 2) * HD
            for qt in range(NQT):
                sps = []
                for j in range(NKB):
                    sp = psS.tile([KB, QT], F32, name="s_ps", tag="qk")
                    nc.tensor.matmul(sp[:], kTt[ch][ds(off, HD), ts(j, KB)],
                                     qTt[ch][ds(off, HD), ts(qt, QT)],
                                     start=True, stop=True)
                    sps.append(sp)
                pts = []
                for j in range(NKB):
                    pt = phC.tile([KB, QT], BF16, name="p_t", tag="probs",
                                  bufs=21)
                    nc.scalar.activation(pt[:], sps[j][:], AF.Exp)
                    pts.append(pt)
                if os.environ.get("K_PIPE", "1") == "1":
                    if prev is not None:
                        _drain(prev)
                    prev = (h, qt, pts)
                else:
                    _drain((h, qt, pts))
        if prev is not None:
            _drain(prev)

    if os.environ.get("K_PHASES", "all") == "c":
        with tc.tile_pool(name="dbg", bufs=1) as dbg:
            dt = dbg.tile([128, NQ], F32, name="dbg_t", tag="dbg_t")
            nc.scalar.activation(dt[:], oT[0][:], AF.Identity)
            nc.sync.dma_start(out_d[0:128, :], dt[:])
        return

    # ---- phase D: output projection, transposed out (host transposes) ----
    # out DMAs write full contiguous row-blocks (partial-row DMA to DRAM
    # breaks profiled execution on this runtime)
    with tc.tile_pool(name="phD", bufs=1) as phD, \
            tc.tile_pool(name="psD", bufs=3, space="PSUM") as psD:
        otf = [phD.tile([128, NQ], F32, name=f"outT{f}", tag=f"outT{f}")
               for f in range(DC)]
        for qt in range(NQT):
            for f in range(DC):
                ps = psD.tile([128, QT], F32, name="ps_o", tag="proj")
                for c in range(DC):
                    nc.tensor.matmul(
                        ps[:], wp_t[c][:, ts(f, 128)], oT[c][:, ts(qt, QT)],
                        start=(c == 0), stop=(c == DC - 1))
                nc.scalar.activation(otf[f][:, ts(qt, QT)], ps[:], AF.Identity,
                                     bias=pb_t[:, ds(f, 1)])
                if qt == NQT - 1:
                    nc.sync.dma_start(out_d[ts(f, 128), :], otf[f][:])


def build_program():
    from concourse import bacc
    from concourse.compiler_utils import get_compiler_flags, set_compiler_flags
    flags = [f.replace("--enable-ldw-opt=false", "--enable-ldw-opt=true")
             for f in get_compiler_flags()]
    set_compiler_flags(flags)
    nc = bacc.Bacc("TRN2", target_bir_lowering=False, debug=False,
                   num_devices=NCORES)
    mk = lambda name, shape, dt=BF16, out=False: nc.dram_tensor(
        name, shape, dt, kind="ExternalOutput" if out else "ExternalInput").ap()
    ins = [
        mk("qinT", [DIM, NQ]), mk("sT", [DIM, NK]),
        mk("wqT", [DIM, DIM]), mk("wkT", [DIM, DIM]),
        mk("wvT", [DIM, DIM]), mk("wpT", [DIM, DIM]),
        mk("qb2", [128, DC], F32), mk("kb2", [128, DC], F32),
        mk("pb2", [128, DC], F32), mk("vbb", [128, DIM], F32),
    ]
    outs = [mk("out", [DIM, NQ], F32, out=True)]
    with tile.TileContext(nc) as tc:
        with ExitStack() as ctx:
            _emit(ctx, tc, outs, ins)
    nc.compile()
    return nc


def host_prep(inputs):
    """Host-side marshalling: per-core slice, add positional sums, transpose
    to [feature, token], cast to bf16, fold the attention scale into Wq."""
    import ml_dtypes
    BF = ml_dtypes.bfloat16
    f32 = np.float32
    g = {k: np.asarray(v, dtype=f32) for k, v in inputs.items()}
    t_pat = g["t_x"][1:]                      # (VP, B*T, D)
    s_x = g["s_x"]                            # (AP, B*SPEC, D)

    posq = (g["vmae_space_pos"][:, None, :]
            + g["vmae_temporal_pos"][None, :, :]).reshape(NQ, DIM)
    poss = (g["clip_space_pos"][:, None, :]
            + g["clip_temporal_pos"][None, :, :]).reshape(NK, DIM)

    cT = lambda a, dt: np.ascontiguousarray(np.asarray(a, dtype=dt).T)
    wqT = cT(SCALE * g["Wq"], BF)
    wkT = cT(g["Wkv"][:DIM], BF)
    wvT = cT(g["Wkv"][DIM:], BF)
    wpT = cT(g["Wproj"], BF)
    qb2 = np.ascontiguousarray((SCALE * g["q_bias"]).reshape(DC, 128).T)
    kb2 = np.ascontiguousarray(g["kv_bias"][:DIM].reshape(DC, 128).T)
    pb2 = np.ascontiguousarray(g["proj_bias"].reshape(DC, 128).T)
    vbb = np.ascontiguousarray(np.tile(g["kv_bias"][DIM:], (128, 1)))

    shared = dict(wqT=wqT, wkT=wkT, wvT=wvT, wpT=wpT,
                  qb2=qb2, kb2=kb2, pb2=pb2, vbb=vbb)
    in_maps = []
    for b in range(B):
        qinT = cT(t_pat[:, b * T:(b + 1) * T, :].reshape(NQ, DIM) + posq, BF)
        sTn = cT(s_x[:, b * SPEC:(b + 1) * SPEC, :].reshape(NK, DIM) + poss, BF)
        in_maps.append(dict(qinT=qinT, sT=sTn, **shared))
    return in_maps


def host_finish(results, t_x):
    o = np.stack([np.asarray(results[b]["out"], np.float32).T
                  for b in range(B)])                       # (B, NQ, D)
    o = o.reshape(B, VP, T, DIM).transpose(1, 0, 2, 3).reshape(VP, B * T, DIM)
    return np.concatenate([np.asarray(t_x, dtype=np.float32)[0:1], o], axis=0)


_NC = None


def kernel(**inputs):
    global _NC
    from concourse.bass_utils import run_bass_kernel_spmd
    if _NC is None:
        _NC = build_program()
    in_maps = host_prep(inputs)
    res = run_bass_kernel_spmd(_NC, in_maps, list(range(NCORES)))
    return host_finish(res.results, inputs["t_x"])


# revision 16
# speedup vs baseline: 2.3664x; 1.4776x over previous
"""CrossAttentionS2T (attn_all_frame=True) as a Bass/Tile kernel on 8 trn2 cores.

Strategy: data-parallel over batch B=8 -> one batch element per NeuronCore.
All activations live in transposed [feature, token] layout so every matmul
contracts over the partition axis; everything is bf16 (validated 1.3e-4 rel
err end-to-end vs the 2e-2 gate) so DMA volume is halved and the PE runs at
1 cycle/row.

Key performance structure vs the v1 kernel:
  - positional sums are folded into the inputs on the host (q_in, s built
    host-side), removing 7.2MB of DMA and all input-side vector adds
  - Q projection runs first with contraction-outer accumulation over 6 psum
    banks, so the first matmul needs only 1 weight chunk + 1 input chunk
    (~0.8MB of DMA) instead of the whole input
  - softmax denominator comes from 64 replicated ones-columns appended to V
    per head: the PV matmul yields [64 o-rows | 64 denom-rows] in one psum
    tile; normalization is then a 64-lane reciprocal_approx_fast + one
    tensor_mul on DVE, entirely off the PE critical path (v1 used a 1-lane
    reciprocal + PE broadcast matmul per tile = 2.6us serial each)
  - attention is software-pipelined: scores(t+1) are emitted before the
    PV-accumulate(t), so the PE never waits for the Exp activations and the
    HAM clock gate stays at 2.4GHz
  - output is written transposed [feature, token] and the final transpose is
    done on the host (frees ~78 PE transposes + DVE copies)
"""

import math
import os
from contextlib import ExitStack

import numpy as np

import concourse.bass as bass
import concourse.mybir as mybir
import concourse.tile as tile
from concourse.bass import ds, ts

F32 = mybir.dt.float32
BF16 = mybir.dt.bfloat16
AF = mybir.ActivationFunctionType

# problem dims (hardcoded per contract)
B, SPEC, T = 8, 4, 8
AP_, VP, DIM = 196, 196, 768
NH, HD = 12, 64
SCALE = HD ** -0.5
NQ = VP * T          # 1568 q tokens per batch
NK = AP_ * SPEC      # 784 kv tokens per batch
DC = DIM // 128      # 6 contraction chunks
QT, NQT = 392, 4     # q-token tile (moving free dim)
KB, NKB = 112, 7     # k-token block (scores.T partition dim)
VW, NVW = 384, 2     # v feature tile for natural-layout V projection
VH = 128             # per-head stride in v tiles: 64 v cols + 64 ones cols
NCORES = 8


def _emit(ctx, tc, outs, ins):
    nc = tc.nc
    (qinT_d, sT_d, wqT_d, wkT_d, wvT_d, wpT_d, qb2, kb2, pb2, vbb) = ins
    (out_d,) = outs

    const = ctx.enter_context(tc.tile_pool(name="const", bufs=1))
    qb_t = const.tile([128, DC], F32)
    kb_t = const.tile([128, DC], F32)
    pb_t = const.tile([128, DC], F32)
    vbb_t = const.tile([128, DIM], F32)
    nc.sync.dma_start(qb_t[:], qb2[:])
    nc.sync.dma_start(kb_t[:], kb2[:])
    nc.sync.dma_start(pb_t[:], pb2[:])
    nc.sync.dma_start(vbb_t[:], vbb[:])

    # persistent activations (all bf16). q/k live per-head in [128, tok]
    # tiles with rows 64-127 zeroed, so the scores matmul contracts over
    # K=128 (K=64 runs the PE at half rate).
    pers = ctx.enter_context(tc.tile_pool(name="pers", bufs=1))
    qH = [pers.tile([128, NQ], BF16, name=f"qH{h}", tag=f"qH{h}")
          for h in range(NH)]
    kH = [pers.tile([128, NK], BF16, name=f"kH{h}", tag=f"kH{h}")
          for h in range(NH)]
    v_t = [pers.tile([KB, NH * VH], BF16, name=f"v{j}", tag=f"v{j}")
           for j in range(NKB)]
    oT = [pers.tile([128, NQ], BF16, name=f"oT{c}", tag=f"oT{c}")
          for c in range(DC)]
    wp_t = [pers.tile([128, DIM], BF16, name=f"wp{c}", tag=f"wp{c}")
            for c in range(DC)]

    # ones columns of v tiles (denominator trick): v_t[j][:, h*128+64 : +128]
    for j in range(NKB):
        for h in range(NH):
            nc.vector.memset(v_t[j][:, ds(h * VH + HD, HD)], 1.0)
    # zero pad rows of per-head q/k tiles
    for h in range(NH):
        nc.vector.memset(qH[h][ds(HD, HD), :], 0.0)
        nc.vector.memset(kH[h][ds(HD, HD), :], 0.0)

    with tc.tile_pool(name="phKV", bufs=1) as phKV:
        wk_t = [phKV.tile([128, DIM], BF16, name=f"wk{c}", tag=f"wk{c}")
                for c in range(DC)]
        wv_t = [phKV.tile([128, DIM], BF16, name=f"wv{c}", tag=f"wv{c}")
                for c in range(DC)]
        sTn = [phKV.tile([128, NK], BF16, name=f"sT{c}", tag=f"sT{c}")
               for c in range(DC)]
        # KV-side DMAs go on the gpsimd queue, in parallel with the Q-side
        # stream on the sync queue
        for c in range(DC):
            nc.gpsimd.dma_start(wk_t[c][:], wkT_d[ts(c, 128), :])
            nc.gpsimd.dma_start(sTn[c][:], sT_d[ts(c, 128), :])
        for c in range(DC):
            nc.gpsimd.dma_start(wv_t[c][:], wvT_d[ts(c, 128), :])

        # ---- phase Q: Q projection, contraction-outer for fast PE start ----
        with tc.tile_pool(name="phQ", bufs=1) as phQ, \
                tc.tile_pool(name="psQ", bufs=6, space="PSUM") as psQ:
            wq_t = [phQ.tile([128, DIM], BF16, name=f"wq{c}", tag=f"wq{c}")
                    for c in range(DC)]
            qin = [phQ.tile([128, NQ], BF16, name=f"qi{c}", tag=f"qi{c}")
                   for c in range(DC)]
            for c in range(DC):
                nc.sync.dma_start(wq_t[c][:], wqT_d[ts(c, 128), :])
                nc.sync.dma_start(qin[c][:], qinT_d[ts(c, 128), :])
            for c in range(DC):
                nc.sync.dma_start(wp_t[c][:], wpT_d[ts(c, 128), :])
            for qt in range(NQT):
                pss = [psQ.tile([128, QT], F32, name="ps_q", tag="q")
                       for _ in range(DC)]
                for c in range(DC):
                    for f in range(DC):
                        nc.tensor.matmul(
                            pss[f][:], wq_t[c][:, ts(f, 128)],
                            qin[c][:, ts(qt, QT)],
                            start=(c == 0), stop=(c == DC - 1))
                for f in range(DC):
                    nc.scalar.activation(qH[2 * f][ds(0, HD), ts(qt, QT)],
                                         pss[f][ds(0, HD), :], AF.Identity,
                                         bias=qb_t[ds(0, HD), ds(f, 1)])
                    nc.scalar.activation(qH[2 * f + 1][ds(0, HD), ts(qt, QT)],
                                         pss[f][ds(HD, HD), :], AF.Identity,
                                         bias=qb_t[ds(HD, HD), ds(f, 1)])

        # ---- phase KV: K (transposed) and V (natural + ones) projections ----
        with tc.tile_pool(name="psKV", bufs=6, space="PSUM") as psKV:
            for kt in range(2):
                psk = [psKV.tile([128, QT], F32, name="ps_k", tag="kv")
                       for _ in range(DC)]
                for c in range(DC):
                    for f in range(DC):
                        nc.tensor.matmul(
                            psk[f][:], wk_t[c][:, ts(f, 128)],
                            sTn[c][:, ts(kt, QT)],
                            start=(c == 0), stop=(c == DC - 1))
                for f in range(DC):
                    nc.scalar.activation(kH[2 * f][ds(0, HD), ts(kt, QT)],
                                         psk[f][ds(0, HD), :], AF.Identity,
                                         bias=kb_t[ds(0, HD), ds(f, 1)])
                    nc.scalar.activation(kH[2 * f + 1][ds(0, HD), ts(kt, QT)],
                                         psk[f][ds(HD, HD), :], AF.Identity,
                                         bias=kb_t[ds(HD, HD), ds(f, 1)])
            for j in range(NKB):
                for w in range(NVW):
                    ps = psKV.tile([KB, VW], F32, name="ps_v", tag="kv")
                    for c in range(DC):
                        nc.tensor.matmul(
                            ps[:], sTn[c][:, ts(j, KB)], wv_t[c][:, ts(w, VW)],
                            start=(c == 0), stop=(c == DC - 1))
                    with nc.allow_low_precision(reason="bf16 v store"):
                        for hh in range(6):
                            h = w * 6 + hh
                            nc.vector.tensor_add(
                                v_t[j][:, ds(h * VH, HD)],
                                ps[:, ts(hh, HD)],
                                vbb_t[0:KB, ds(w * VW + hh * HD, HD)])

    if os.environ.get("K_PHASES", "all") == "qkv":
        with tc.tile_pool(name="dbg", bufs=1) as dbg:
            dt = dbg.tile([128, NK], F32, name="dbg_t", tag="dbg_t")
            nc.scalar.activation(dt[:], kH[0][:], AF.Identity)
            nc.sync.dma_start(out_d[0:128, 0:NK], dt[:])
        return

    # ---- phase C: attention, software-pipelined ----
    with tc.tile_pool(name="phC", bufs=1) as phC, \
            tc.tile_pool(name="psS", bufs=6, space="PSUM") as psS, \
            tc.tile_pool(name="psO", bufs=2, space="PSUM") as psO:

        def _drain(prev):
            h, qt, pts = prev
            ch, off = h // 2, (h % 2) * HD
            op = psO.tile([128, QT], F32, name="o_ps", tag="o")
            for j in range(NKB):
                nc.tensor.matmul(op[:], v_t[j][:, ds(h * VH, VH)], pts[j][:],
                                 start=(j == 0), stop=(j == NKB - 1))
            scr = phC.tile([HD, QT], F32, name="scr", tag="scr", bufs=2)
            nc.vector.reciprocal(scr[:], op[ds(HD, HD), :])
            with nc.allow_low_precision(reason="bf16 attn out"):
                nc.vector.tensor_mul(oT[ch][ds(off, HD), ts(qt, QT)],
                                     op[ds(0, HD), :], scr[:])

        prev = None
        for h in range(NH):
            for qt in range(NQT):
                sps = []
                for j in range(NKB):
                    sp = psS.tile([KB, QT], F32, name="s_ps", tag="qk")
                    nc.tensor.matmul(sp[:], kH[h][:, ts(j, KB)],
                                     qH[h][:, ts(qt, QT)],
                                     start=True, stop=True)
                    sps.append(sp)
                pts = []
                for j in range(NKB):
                    pt = phC.tile([KB, QT], BF16, name="p_t", tag="probs",
                                  bufs=21)
                    nc.scalar.activation(pt[:], sps[j][:], AF.Exp)
                    pts.append(pt)
                if os.environ.get("K_PIPE", "1") == "1":
                    if prev is not None:
                        _drain(prev)
                    prev = (h, qt, pts)
                else:
                    _drain((h, qt, pts))
        if prev is not None:
            _drain(prev)

    if os.environ.get("K_PHASES", "all") == "c":
        with tc.tile_pool(name="dbg", bufs=1) as dbg:
            dt = dbg.tile([128, NQ], F32, name="dbg_t", tag="dbg_t")
            nc.scalar.activation(dt[:], oT[0][:], AF.Identity)
            nc.sync.dma_start(out_d[0:128, :], dt[:])
        return

    # ---- phase D: output projection, transposed out (host transposes) ----
    # out DMAs write full contiguous row-blocks (partial-row DMA to DRAM
    # breaks profiled execution on this runtime)
    with tc.tile_pool(name="phD", bufs=1) as phD, \
            tc.tile_pool(name="psD", bufs=3, space="PSUM") as psD:
        otf = [phD.tile([128, NQ], F32, name=f"outT{f}", tag=f"outT{f}")
               for f in range(DC)]
        for qt in range(NQT):
            for f in range(DC):
                ps = psD.tile([128, QT], F32, name="ps_o", tag="proj")
                for c in range(DC):
                    nc.tensor.matmul(
                        ps[:], wp_t[c][:, ts(f, 128)], oT[c][:, ts(qt, QT)],
                        start=(c == 0), stop=(c == DC - 1))
                nc.scalar.activation(otf[f][:, ts(qt, QT)], ps[:], AF.Identity,
                                     bias=pb_t[:, ds(f, 1)])
                if qt == NQT - 1:
                    nc.sync.dma_start(out_d[ts(f, 128), :], otf[f][:])


def build_program():
    from concourse import bacc
    from concourse.compiler_utils import get_compiler_flags, set_compiler_flags
    flags = [f.replace("--enable-ldw-opt=false", "--enable-ldw-opt=true")
             for f in get_compiler_flags()]
    set_compiler_flags(flags)
    nc = bacc.Bacc("TRN2", target_bir_lowering=False, debug=False,
                   num_devices=NCORES)
    mk = lambda name, shape, dt=BF16, out=False: nc.dram_tensor(
        name, shape, dt, kind="ExternalOutput" if out else "ExternalInput").ap()
    ins = [
        mk("qinT", [DIM, NQ]), mk("sT", [DIM, NK]),
        mk("wqT", [DIM, DIM]), mk("wkT", [DIM, DIM]),
        mk("wvT", [DIM, DIM]), mk("wpT", [DIM, DIM]),
        mk("qb2", [128, DC], F32), mk("kb2", [128, DC], F32),
        mk("pb2", [128, DC], F32), mk("vbb", [128, DIM], F32),
    ]
    outs = [mk("out", [DIM, NQ], F32, out=True)]
    with tile.TileContext(nc) as tc:
        with ExitStack() as ctx:
            _emit(ctx, tc, outs, ins)
    nc.compile()
    return nc


def host_prep(inputs):
    """Host-side marshalling: per-core slice, add positional sums, transpose
    to [feature, token], cast to bf16, fold the attention scale into Wq."""
    import ml_dtypes
    BF = ml_dtypes.bfloat16
    f32 = np.float32
    g = {k: np.asarray(v, dtype=f32) for k, v in inputs.items()}
    t_pat = g["t_x"][1:]                      # (VP, B*T, D)
    s_x = g["s_x"]                            # (AP, B*SPEC, D)

    posq = (g["vmae_space_pos"][:, None, :]
            + g["vmae_temporal_pos"][None, :, :]).reshape(NQ, DIM)
    poss = (g["clip_space_pos"][:, None, :]
            + g["clip_temporal_pos"][None, :, :]).reshape(NK, DIM)

    cT = lambda a, dt: np.ascontiguousarray(np.asarray(a, dtype=dt).T)
    wqT = cT(SCALE * g["Wq"], BF)
    wkT = cT(g["Wkv"][:DIM], BF)
    wvT = cT(g["Wkv"][DIM:], BF)
    wpT = cT(g["Wproj"], BF)
    qb2 = np.ascontiguousarray((SCALE * g["q_bias"]).reshape(DC, 128).T)
    kb2 = np.ascontiguousarray(g["kv_bias"][:DIM].reshape(DC, 128).T)
    pb2 = np.ascontiguousarray(g["proj_bias"].reshape(DC, 128).T)
    vbb = np.ascontiguousarray(np.tile(g["kv_bias"][DIM:], (128, 1)))

    shared = dict(wqT=wqT, wkT=wkT, wvT=wvT, wpT=wpT,
                  qb2=qb2, kb2=kb2, pb2=pb2, vbb=vbb)
    in_maps = []
    for b in range(B):
        qinT = cT(t_pat[:, b * T:(b + 1) * T, :].reshape(NQ, DIM) + posq, BF)
        sTn = cT(s_x[:, b * SPEC:(b + 1) * SPEC, :].reshape(NK, DIM) + poss, BF)
        in_maps.append(dict(qinT=qinT, sT=sTn, **shared))
    return in_maps


def host_finish(results, t_x):
    o = np.stack([np.asarray(results[b]["out"], np.float32).T
                  for b in range(B)])                       # (B, NQ, D)
    o = o.reshape(B, VP, T, DIM).transpose(1, 0, 2, 3).reshape(VP, B * T, DIM)
    return np.concatenate([np.asarray(t_x, dtype=np.float32)[0:1], o], axis=0)


_NC = None


def kernel(**inputs):
    global _NC
    from concourse.bass_utils import run_bass_kernel_spmd
    if _NC is None:
        _NC = build_program()
    in_maps = host_prep(inputs)
    res = run_bass_kernel_spmd(_NC, in_maps, list(range(NCORES)))
    return host_finish(res.results, inputs["t_x"])


# revision 18
# speedup vs baseline: 2.3944x; 1.0118x over previous
"""CrossAttentionS2T (attn_all_frame=True) as a Bass/Tile kernel on 8 trn2 cores.

Strategy: data-parallel over batch B=8 -> one batch element per NeuronCore.
All activations live in transposed [feature, token] layout so every matmul
contracts over the partition axis; everything is bf16 (validated 1.3e-4 rel
err end-to-end vs the 2e-2 gate) so DMA volume is halved and the PE runs at
1 cycle/row.

Key performance structure vs the v1 kernel:
  - positional sums are folded into the inputs on the host (q_in, s built
    host-side), removing 7.2MB of DMA and all input-side vector adds
  - Q projection runs first with contraction-outer accumulation over 6 psum
    banks, so the first matmul needs only 1 weight chunk + 1 input chunk
    (~0.8MB of DMA) instead of the whole input
  - softmax denominator comes from 64 replicated ones-columns appended to V
    per head: the PV matmul yields [64 o-rows | 64 denom-rows] in one psum
    tile; normalization is then a 64-lane reciprocal_approx_fast + one
    tensor_mul on DVE, entirely off the PE critical path (v1 used a 1-lane
    reciprocal + PE broadcast matmul per tile = 2.6us serial each)
  - attention is software-pipelined: scores(t+1) are emitted before the
    PV-accumulate(t), so the PE never waits for the Exp activations and the
    HAM clock gate stays at 2.4GHz
  - output is written transposed [feature, token] and the final transpose is
    done on the host (frees ~78 PE transposes + DVE copies)
"""

import math
import os
from contextlib import ExitStack

import numpy as np

import concourse.bass as bass
import concourse.mybir as mybir
import concourse.tile as tile
from concourse.bass import ds, ts

F32 = mybir.dt.float32
BF16 = mybir.dt.bfloat16
AF = mybir.ActivationFunctionType

# problem dims (hardcoded per contract)
B, SPEC, T = 8, 4, 8
AP_, VP, DIM = 196, 196, 768
NH, HD = 12, 64
SCALE = HD ** -0.5
NQ = VP * T          # 1568 q tokens per batch
NK = AP_ * SPEC      # 784 kv tokens per batch
DC = DIM // 128      # 6 contraction chunks
QT, NQT = 392, 4     # q-token tile (moving free dim)
KB, NKB = 112, 7     # k-token block (scores.T partition dim)
VW, NVW = 384, 2     # v feature tile for natural-layout V projection
VH = 128             # per-head stride in v tiles: 64 v cols + 64 ones cols
NCORES = 8


def _emit(ctx, tc, outs, ins):
    nc = tc.nc
    (qinT_d, sT_d, wqT_d, wkT_d, wvT_d, wpT_d, qb2, kb2, pb2, vbb) = ins
    (out_d,) = outs

    const = ctx.enter_context(tc.tile_pool(name="const", bufs=1))
    qb_t = const.tile([128, DC], F32)
    kb_t = const.tile([128, DC], F32)
    pb_t = const.tile([128, DC], F32)
    vbb_t = const.tile([128, DIM], F32)

    # persistent activations (all bf16). q/k live per-head in [128, tok]
    # tiles with rows 64-127 zeroed, so the scores matmul contracts over
    # K=128 (K=64 runs the PE at half rate).
    pers = ctx.enter_context(tc.tile_pool(name="pers", bufs=1))
    qH = [pers.tile([128, NQ], BF16, name=f"qH{h}", tag=f"qH{h}")
          for h in range(NH)]
    kH = [pers.tile([128, NK], BF16, name=f"kH{h}", tag=f"kH{h}")
          for h in range(NH)]
    v_t = [pers.tile([KB, NH * VH], BF16, name=f"v{j}", tag=f"v{j}")
           for j in range(NKB)]
    oT = [pers.tile([128, NQ], BF16, name=f"oT{c}", tag=f"oT{c}")
          for c in range(DC)]
    wp_t = [pers.tile([128, DIM], BF16, name=f"wp{c}", tag=f"wp{c}")
            for c in range(DC)]

    # ones columns of v tiles (denominator trick): v_t[j][:, h*128+64 : +128]
    for j in range(NKB):
        for h in range(NH):
            nc.vector.memset(v_t[j][:, ds(h * VH + HD, HD)], 1.0)
    # zero pad rows of per-head q/k tiles
    for h in range(NH):
        nc.vector.memset(qH[h][ds(HD, HD), :], 0.0)
        nc.vector.memset(kH[h][ds(HD, HD), :], 0.0)

    with tc.tile_pool(name="phKV", bufs=1) as phKV:
        wk_t = [phKV.tile([128, DIM], BF16, name=f"wk{c}", tag=f"wk{c}")
                for c in range(DC)]
        wv_t = [phKV.tile([128, DIM], BF16, name=f"wv{c}", tag=f"wv{c}")
                for c in range(DC)]
        sTn = [phKV.tile([128, NK], BF16, name=f"sT{c}", tag=f"sT{c}")
               for c in range(DC)]
        # KV-side DMAs go on the gpsimd queue, in parallel with the Q-side
        # stream on the sync queue
        for c in range(DC):
            nc.gpsimd.dma_start(wk_t[c][:], wkT_d[ts(c, 128), :])
            nc.gpsimd.dma_start(sTn[c][:], sT_d[ts(c, 128), :])
        for c in range(DC):
            nc.gpsimd.dma_start(wv_t[c][:], wvT_d[ts(c, 128), :])

        # ---- phase Q: Q projection, contraction-outer for fast PE start ----
        with tc.tile_pool(name="phQ", bufs=1) as phQ, \
                tc.tile_pool(name="psQ", bufs=6, space="PSUM") as psQ:
            wq_t = [phQ.tile([128, DIM], BF16, name=f"wq{c}", tag=f"wq{c}")
                    for c in range(DC)]
            qin = [phQ.tile([128, NQ], BF16, name=f"qi{c}", tag=f"qi{c}")
                   for c in range(DC)]
            # critical-path DMAs first: the first matmul needs only wq0+qin0
            for c in range(DC):
                nc.sync.dma_start(wq_t[c][:], wqT_d[ts(c, 128), :])
                nc.sync.dma_start(qin[c][:], qinT_d[ts(c, 128), :])
            nc.sync.dma_start(qb_t[:], qb2[:])
            nc.sync.dma_start(kb_t[:], kb2[:])
            nc.sync.dma_start(pb_t[:], pb2[:])
            nc.gpsimd.dma_start(vbb_t[:], vbb[:])
            for c in range(DC):
                nc.sync.dma_start(wp_t[c][:], wpT_d[ts(c, 128), :])
            for qt in range(NQT):
                pss = [psQ.tile([128, QT], F32, name="ps_q", tag="q")
                       for _ in range(DC)]
                for c in range(DC):
                    for f in range(DC):
                        nc.tensor.matmul(
                            pss[f][:], wq_t[c][:, ts(f, 128)],
                            qin[c][:, ts(qt, QT)],
                            start=(c == 0), stop=(c == DC - 1))
                for f in range(DC):
                    nc.scalar.activation(qH[2 * f][ds(0, HD), ts(qt, QT)],
                                         pss[f][ds(0, HD), :], AF.Identity,
                                         bias=qb_t[ds(0, HD), ds(f, 1)])
                    nc.scalar.activation(qH[2 * f + 1][ds(0, HD), ts(qt, QT)],
                                         pss[f][ds(HD, HD), :], AF.Identity,
                                         bias=qb_t[ds(HD, HD), ds(f, 1)])

        # ---- phase KV: K (transposed) and V (natural + ones) projections ----
        with tc.tile_pool(name="psKV", bufs=6, space="PSUM") as psKV:
            for kt in range(2):
                psk = [psKV.tile([128, QT], F32, name="ps_k", tag="kv")
                       for _ in range(DC)]
                for c in range(DC):
                    for f in range(DC):
                        nc.tensor.matmul(
                            psk[f][:], wk_t[c][:, ts(f, 128)],
                            sTn[c][:, ts(kt, QT)],
                            start=(c == 0), stop=(c == DC - 1))
                for f in range(DC):
                    nc.scalar.activation(kH[2 * f][ds(0, HD), ts(kt, QT)],
                                         psk[f][ds(0, HD), :], AF.Identity,
                                         bias=kb_t[ds(0, HD), ds(f, 1)])
                    nc.scalar.activation(kH[2 * f + 1][ds(0, HD), ts(kt, QT)],
                                         psk[f][ds(HD, HD), :], AF.Identity,
                                         bias=kb_t[ds(HD, HD), ds(f, 1)])
            for j in range(NKB):
                for w in range(NVW):
                    ps = psKV.tile([KB, VW], F32, name="ps_v", tag="kv")
                    for c in range(DC):
                        nc.tensor.matmul(
                            ps[:], sTn[c][:, ts(j, KB)], wv_t[c][:, ts(w, VW)],
                            start=(c == 0), stop=(c == DC - 1))
                    with nc.allow_low_precision(reason="bf16 v store"):
                        for hh in range(6):
                            h = w * 6 + hh
                            nc.vector.tensor_add(
                                v_t[j][:, ds(h * VH, HD)],
                                ps[:, ts(hh, HD)],
                                vbb_t[0:KB, ds(w * VW + hh * HD, HD)])

    if os.environ.get("K_PHASES", "all") == "qkv":
        with tc.tile_pool(name="dbg", bufs=1) as dbg:
            dt = dbg.tile([128, NK], F32, name="dbg_t", tag="dbg_t")
            nc.scalar.activation(dt[:], kH[0][:], AF.Identity)
            nc.sync.dma_start(out_d[0:128, 0:NK], dt[:])
        return

    # ---- phase C: attention, software-pipelined ----
    with tc.tile_pool(name="phC", bufs=1) as phC, \
            tc.tile_pool(name="psS", bufs=6, space="PSUM") as psS, \
            tc.tile_pool(name="psO", bufs=2, space="PSUM") as psO:

        def _drain(prev):
            h, qt, pts = prev
            ch, off = h // 2, (h % 2) * HD
            op = psO.tile([128, QT], F32, name="o_ps", tag="o")
            for j in range(NKB):
                nc.tensor.matmul(op[:], v_t[j][:, ds(h * VH, VH)], pts[j][:],
                                 start=(j == 0), stop=(j == NKB - 1))
            scr = phC.tile([HD, QT], F32, name="scr", tag="scr", bufs=2)
            nc.vector.reciprocal(scr[:], op[ds(HD, HD), :])
            with nc.allow_low_precision(reason="bf16 attn out"):
                nc.vector.tensor_mul(oT[ch][ds(off, HD), ts(qt, QT)],
                                     op[ds(0, HD), :], scr[:])

        prev = None
        for h in range(NH):
            for qt in range(NQT):
                sps = []
                for j in range(NKB):
                    sp = psS.tile([KB, QT], F32, name="s_ps", tag="qk")
                    nc.tensor.matmul(sp[:], kH[h][:, ts(j, KB)],
                                     qH[h][:, ts(qt, QT)],
                                     start=True, stop=True)
                    sps.append(sp)
                pts = []
                for j in range(NKB):
                    pt = phC.tile([KB, QT], BF16, name="p_t", tag="probs",
                                  bufs=21)
                    nc.scalar.activation(pt[:], sps[j][:], AF.Exp)
                    pts.append(pt)
                if os.environ.get("K_PIPE", "1") == "1":
                    if prev is not None:
                        _drain(prev)
                    prev = (h, qt, pts)
                else:
                    _drain((h, qt, pts))
        if prev is not None:
            _drain(prev)

    if os.environ.get("K_PHASES", "all") == "c":
        with tc.tile_pool(name="dbg", bufs=1) as dbg:
            dt = dbg.tile([128, NQ], F32, name="dbg_t", tag="dbg_t")
            nc.scalar.activation(dt[:], oT[0][:], AF.Identity)
            nc.sync.dma_start(out_d[0:128, :], dt[:])
        return

    # ---- phase D: output projection, transposed out (host transposes) ----
    # out DMAs write full contiguous row-blocks (partial-row DMA to DRAM
    # breaks profiled execution on this runtime)
    with tc.tile_pool(name="phD", bufs=1) as phD, \
            tc.tile_pool(name="psD", bufs=3, space="PSUM") as psD:
        otf = [phD.tile([128, NQ], F32, name=f"outT{f}", tag=f"outT{f}")
               for f in range(DC)]
        for qt in range(NQT):
            for f in range(DC):
                ps = psD.tile([128, QT], F32, name="ps_o", tag="proj")
                for c in range(DC):
                    nc.tensor.matmul(
                        ps[:], wp_t[c][:, ts(f, 128)], oT[c][:, ts(qt, QT)],
                        start=(c == 0), stop=(c == DC - 1))
                nc.scalar.activation(otf[f][:, ts(qt, QT)], ps[:], AF.Identity,
                                     bias=pb_t[:, ds(f, 1)])
                if qt == NQT - 1:
                    nc.sync.dma_start(out_d[ts(f, 128), :], otf[f][:])


def build_program():
    from concourse import bacc
    from concourse.compiler_utils import get_compiler_flags, set_compiler_flags
    flags = [f.replace("--enable-ldw-opt=false", "--enable-ldw-opt=true")
             for f in get_compiler_flags()]
    set_compiler_flags(flags)
    nc = bacc.Bacc("TRN2", target_bir_lowering=False, debug=False,
                   num_devices=NCORES)
    mk = lambda name, shape, dt=BF16, out=False: nc.dram_tensor(
        name, shape, dt, kind="ExternalOutput" if out else "ExternalInput").ap()
    ins = [
        mk("qinT", [DIM, NQ]), mk("sT", [DIM, NK]),
        mk("wqT", [DIM, DIM]), mk("wkT", [DIM, DIM]),
        mk("wvT", [DIM, DIM]), mk("wpT", [DIM, DIM]),
        mk("qb2", [128, DC], F32), mk("kb2", [128, DC], F32),
        mk("pb2", [128, DC], F32), mk("vbb", [128, DIM], F32),
    ]
    outs = [mk("out", [DIM, NQ], F32, out=True)]
    with tile.TileContext(nc) as tc:
        with ExitStack() as ctx:
            _emit(ctx, tc, outs, ins)
    nc.compile()
    return nc


def host_prep(inputs):
    """Host-side marshalling: per-core slice, add positional sums, transpose
    to [feature, token], cast to bf16, fold the attention scale into Wq."""
    import ml_dtypes
    BF = ml_dtypes.bfloat16
    f32 = np.float32
    g = {k: np.asarray(v, dtype=f32) for k, v in inputs.items()}
    t_pat = g["t_x"][1:]                      # (VP, B*T, D)
    s_x = g["s_x"]                            # (AP, B*SPEC, D)

    posq = (g["vmae_space_pos"][:, None, :]
            + g["vmae_temporal_pos"][None, :, :]).reshape(NQ, DIM)
    poss = (g["clip_space_pos"][:, None, :]
            + g["clip_temporal_pos"][None, :, :]).reshape(NK, DIM)

    cT = lambda a, dt: np.ascontiguousarray(np.asarray(a, dtype=dt).T)
    wqT = cT(SCALE * g["Wq"], BF)
    wkT = cT(g["Wkv"][:DIM], BF)
    wvT = cT(g["Wkv"][DIM:], BF)
    wpT = cT(g["Wproj"], BF)
    qb2 = np.ascontiguousarray((SCALE * g["q_bias"]).reshape(DC, 128).T)
    kb2 = np.ascontiguousarray(g["kv_bias"][:DIM].reshape(DC, 128).T)
    pb2 = np.ascontiguousarray(g["proj_bias"].reshape(DC, 128).T)
    vbb = np.ascontiguousarray(np.tile(g["kv_bias"][DIM:], (128, 1)))

    shared = dict(wqT=wqT, wkT=wkT, wvT=wvT, wpT=wpT,
                  qb2=qb2, kb2=kb2, pb2=pb2, vbb=vbb)
    in_maps = []
    for b in range(B):
        qinT = cT(t_pat[:, b * T:(b + 1) * T, :].reshape(NQ, DIM) + posq, BF)
        sTn = cT(s_x[:, b * SPEC:(b + 1) * SPEC, :].reshape(NK, DIM) + poss, BF)
        in_maps.append(dict(qinT=qinT, sT=sTn, **shared))
    return in_maps


def host_finish(results, t_x):
    o = np.stack([np.asarray(results[b]["out"], np.float32).T
                  for b in range(B)])                       # (B, NQ, D)
    o = o.reshape(B, VP, T, DIM).transpose(1, 0, 2, 3).reshape(VP, B * T, DIM)
    return np.concatenate([np.asarray(t_x, dtype=np.float32)[0:1], o], axis=0)


_NC = None


def kernel(**inputs):
    global _NC
    from concourse.bass_utils import run_bass_kernel_spmd
    if _NC is None:
        _NC = build_program()
    in_maps = host_prep(inputs)
    res = run_bass_kernel_spmd(_NC, in_maps, list(range(NCORES)))
    return host_finish(res.results, inputs["t_x"])
